# revision 32
# baseline (speedup 1.0000x reference)
"""AttentionPooling (ragged segment attention) on 8 Trainium2 NeuronCores.

Full inputs in, full output out. Strategy (data-parallel over graphs):
  - 128 graphs are LPT-balanced 16-per-core across 8 cores; each core gets
    its graphs' node embeddings (zero-padded to a multiple of 128 rows).
  - The single shared query is a model parameter, so the q-side is constant-
    folded on the host:  qk8[e,h] = sum_d q_scaled[h,d]*k_w[h*64+d,e].
  - On device (per core), per 512-node group (cols c = h*16 + s):
      s8T[h, n]  = sum_e qk8[e,h] * embT[e,n]      (PE fp8, qk stationary)
      e8T[h, n]  = exp(s8T / QK_SCALE)             (ACT)
      e8[n, h]   = PE-transpose(e8T)               (4x [8,128] -> [128,8])
      em[n, c]   = e8[n, h] * indT[n, s]           (DVE broadcast-mul)
      pooled[c,0:776] += em^T @ [emb | 1 | 0pad]   (PE accum; col 768=colsum)
    Softmax exp(qb) factors cancel between numerator and denominator; the
    phantom-slot correction is a host-computed count added to the colsum.
  - Tail: normalize by colsum, PE-transpose pooled, block-diag v-proj,
    out-proj.  Host gathers the 8x[16,512] results back to [bs, 512].
"""

import numpy as np
import ml_dtypes

BF16 = ml_dtypes.bfloat16
FP8 = ml_dtypes.float8_e3m4
QK_SCALE = 128.0
E = 768
EW = 776            # emb row + ones col + pad  (EW-768-1 zeros)
D = 512
H = 8
DH = 64
NCORES = 8
SLOTS = 16          # graphs per core
COLS = 128          # H * SLOTS
ES = E // 128       # 6 E-slices of 128
EMB_FP8 = True                # pool rhs in fp8 e3m4 (halves the emb stream)
ETB = 6 * 128                 # embT bytes per chunk per partition
EMB_B = EW if EMB_FP8 else EW * 2     # emb bytes per chunk per partition
EBB = EMB_B + SLOTS * 2       # emb+ind bytes per chunk per partition

_prog_cache = {}


def _jlist(nch):
    """Chunks per group: small groups at both ends for fast pipeline
    fill (first scores start after a 98KB DMA, not 393KB) and drain."""
    if nch <= 7:
        js = []
        r = nch
        while r > 0:
            j = min(2, r)
            js.append(j)
            r -= j
        return js
    js = [1, 1, 2]
    rem = nch - 7
    js += [4] * (rem // 4)
    if rem % 4:
        js.append(rem % 4)
    js += [2, 1]
    return js


def _build_program(nch):
    import concourse.bacc as bacc
    import concourse.tile as tile
    import concourse.mybir as mybir
    from concourse.bass import AP

    f32 = mybir.dt.float32
    bf16 = mybir.dt.bfloat16
    f8 = mybir.dt.float8e3
    u8 = mybir.dt.uint8
    AF = mybir.ActivationFunctionType

    J_of = _jlist(nch)
    ngrp = len(J_of)
    ch0 = np.concatenate([[0], np.cumsum(J_of)]).astype(int)  # first chunk of g

    nc = bacc.Bacc(None, target_bir_lowering=False)

    # One contiguous DRAM param per group (sequential HBM reads):
    #   [embT fp8 [s(6), j(J), n(128)] @ p=e%128 | emb fp8 [j, EW] +
    #    indT bf16 [j, 16] @ p=n%128]
    # Group 0 carries a 336B header: qk8 fp8 48B | ident bf16 256B |
    # ones16 bf16 32B -- one DMA covers all launch-critical bytes.
    HDR = 336
    g_d = [nc.declare_dram_parameter(
        f"g{g}", [128, (HDR if g == 0 else 0) + J_of[g] * (ETB + EBB)],
        u8, isOutput=False) for g in range(ngrp)]
    # consts2: vT bf16 [6,512]=6144B | owT bf16 [4,512]=4096B | ph f32 4B |
    #          ob row f32 2048B (partition 0 only)
    c2_d = nc.declare_dram_parameter("c2", [128, 6144 + 4096 + 4 + 2048], u8,
                                     isOutput=False)
    out_d = nc.declare_dram_parameter("out", [SLOTS, D], f32, isOutput=True)

    def bview(tile_, byte_off, dt, shape, nparts=128):
        """AP viewing bytes [byte_off:] of a uint8 tile as dtype with the
        given free-dim shape (row-major, contiguous)."""
        esz = mybir.dt.size(dt)
        assert byte_off % esz == 0
        base = tile_[:, :].bitcast(dt)
        strides = []
        acc = 1
        for s in reversed(shape):
            strides.append(acc)
            acc *= s
        strides = strides[::-1]
        newap = [[base.ap[0][0], nparts]] + [[st, sz]
                                             for st, sz in zip(strides, shape)]
        return AP(base.tensor, base.offset + byte_off // esz, newap)

    def sub(ap, elem_off, shape, nparts=None):
        """Sub-AP at elem_off (in ap dtype elements) with contiguous shape."""
        strides = []
        acc = 1
        for s in reversed(shape):
            strides.append(acc)
            acc *= s
        strides = strides[::-1]
        p = [ap.ap[0][0], nparts if nparts is not None else ap.ap[0][1]]
        newap = [p] + [[st, sz] for st, sz in zip(strides, shape)]
        return AP(ap.tensor, ap.offset + elem_off, newap)

    with tile.TileContext(nc) as tc:
        with (
            tc.tile_pool(name="const", bufs=1) as const,
            tc.tile_pool(name="gb_p", bufs=6) as gb_p,
            tc.tile_pool(name="e8_p", bufs=2) as e8_p,
            tc.tile_pool(name="em_p", bufs=3) as em_p,
            tc.tile_pool(name="small", bufs=1) as small,
            tc.tile_pool(name="ps8", bufs=2, space="PSUM") as ps8,
            tc.tile_pool(name="pse", bufs=2, space="PSUM") as pse,
            tc.tile_pool(name="pacc", bufs=1, space="PSUM") as pacc,
            tc.tile_pool(name="pst", bufs=2, space="PSUM") as pst,
        ):
            # ---- group 0 (with header) goes through the const pool ----
            g0_sb = const.tile([128, HDR + J_of[0] * (ETB + EBB)], u8)
            nc.sync.dma_start(out=g0_sb, in_=g_d[0][:, :])
            qk_v = bview(g0_sb, 0, f8, [ES * 8])
            id_v = bview(g0_sb, 48, bf16, [128])
            id8 = sub(id_v, 0, [8], nparts=8)
            ones16 = bview(g0_sb, 304, bf16, [SLOTS], nparts=1)

            # persistent accumulator: pooled [COLS, 776] f32 (col 768 = colsum)
            ps_pool = pacc.tile([COLS, EW], f32)

            gbs = {0: (g0_sb, HDR)}

            def load_g(g):
                J = J_of[g]
                t = gb_p.tile([128, 4 * (ETB + EBB)], u8, tag="gb")
                nc.sync.dma_start(out=t[:, 0:J * (ETB + EBB)], in_=g_d[g][:, :])
                gbs[g] = (t, 0)

            def emit_scores(g):
                """s8T[h, J*128] = sum_s qk[:,s,:].T @ et[:, s, :]; then exp."""
                J = J_of[g]
                gt, gb0 = gbs[g]
                et = bview(gt, gb0, f8, [ES, J * 128])
                ps_s = ps8.tile([8, 512], f32, tag="s8")
                for s in range(ES):
                    nc.tensor.matmul(ps_s[:, 0:J * 128],
                                     lhsT=sub(qk_v, s * 8, [8]),
                                     rhs=sub(et, s * J * 128, [J * 128]),
                                     start=(s == 0), stop=(s == ES - 1))
                e8T = e8_p.tile([8, 512], bf16, tag="e8")
                nc.scalar.activation(out=e8T[:, 0:J * 128], in_=ps_s[:, 0:J * 128],
                                     func=AF.Exp, scale=1.0 / QK_SCALE)
                return e8T

            def emit_em(g, e8T):
                """e8 = transpose(e8T) per chunk; em[n,(h,s)] = e8*indT."""
                J = J_of[g]
                e8_ps = pse.tile([128, 4, 8], bf16, tag="tp")
                for j in range(J):
                    nc.tensor.transpose(e8_ps[:, j, :],
                                        e8T[:, j * 128:(j + 1) * 128], id8)
                em = em_p.tile([128, 4, H, SLOTS], bf16, tag="em")
                gt, gb0 = gbs[g]
                ind = bview(gt, gb0 + J * ETB + EMB_B, bf16, [J, EBB // 2])
                a = e8_ps[:, 0:J, :]
                bc_e8 = AP(a.tensor, a.offset,
                           [list(a.ap[0]), list(a.ap[1]), list(a.ap[2]),
                            [0, SLOTS]])
                bc_ind = AP(ind.tensor, ind.offset,
                            [list(ind.ap[0]), list(ind.ap[1]), [0, H],
                             [1, SLOTS]])
                nc.vector.tensor_mul(em[:, 0:J, :, :], bc_e8, bc_ind)
                return em

            def emit_pool(g, em):
                J = J_of[g]
                edt = f8 if EMB_FP8 else bf16
                esz = 1 if EMB_FP8 else 2
                gt, gb0 = gbs.pop(g)
                assert (gb0 + J * ETB) % esz == 0
                emb = bview(gt, gb0 + J * ETB, edt, [J, EBB // esz])
                for j in range(J):
                    ch = ch0[g] + j
                    st = (ch == 0)
                    sp = (ch == nch - 1)
                    lhsT = em[:, j, :, :]
                    nc.tensor.matmul(ps_pool[:, 512:EW], lhsT=lhsT,
                                     rhs=sub(emb, j * (EBB // esz) + 512,
                                             [EW - 512]),
                                     start=st, stop=sp)
                    nc.tensor.matmul(ps_pool[:, 0:512], lhsT=lhsT,
                                     rhs=sub(emb, j * (EBB // esz), [512]),
                                     start=st, stop=sp)

            # ---- software pipeline ----
            # stages per g: scores(g) | transpose+em(g-1) | pool(g-2)
            for g in range(1, min(3, ngrp)):
                load_g(g)
            c2_sb = const.tile([128, 12292], u8)
            c2_done = False
            e8s, ems = {}, {}
            for g in range(ngrp):
                e8s[g] = emit_scores(g)
                if g >= 1:
                    ems[g - 1] = emit_em(g - 1, e8s.pop(g - 1))
                if g + 3 < ngrp:
                    load_g(g + 3)
                elif not c2_done:
                    nc.sync.dma_start(out=c2_sb, in_=c2_d[:, :])
                    c2_done = True
                if g >= 2:
                    emit_pool(g - 2, ems.pop(g - 2))
            ems[ngrp - 1] = emit_em(ngrp - 1, e8s.pop(ngrp - 1))
            for g in sorted(ems):
                emit_pool(g, ems.pop(g))

            vT_v = bview(c2_sb, 0, bf16, [ES * D])
            owT_v = bview(c2_sb, 6144, bf16, [4 * D])
            ph_v = bview(c2_sb, 10240, f32, [1])
            obr_v = bview(c2_sb, 10244, bf16, [D], nparts=1)

            # dependency-free filler matmuls: keep the PE HAM-warm across the
            # normalize wait so the projection tail runs at full clock
            ps_dum = ps8.tile([8, 512], f32, tag="s8")
            for i in range(12):
                nc.tensor.matmul(ps_dum[:, 0:8],
                                 lhsT=sub(qk_v, 0, [8]),
                                 rhs=sub(qk_v, 8, [8]),
                                 start=(i == 0), stop=(i == 11))

            # ---- normalize: pooled[c, e] / (colsum[c] + phantom[c]);
            # per-slice on alternating engines, PE transpose right behind ----
            cs_sb = small.tile([COLS, 1], f32)
            nc.vector.tensor_add(cs_sb, ps_pool[:, 768:769], ph_v)
            rec_sb = small.tile([COLS, 1], f32)
            nc.vector.reciprocal(rec_sb, cs_sb)
            pooled = small.tile([COLS, E], bf16)
            nc.scalar.activation(out=pooled[:, 0:384], in_=ps_pool[:, 0:384],
                                 func=AF.Copy, scale=rec_sb[:, :])
            nc.vector.tensor_scalar_mul(pooled[:, 384:768],
                                        in0=ps_pool[:, 384:768],
                                        scalar1=rec_sb)
            # ---- pooledT via PE transposes, 2 PSUM tiles, 2 copies ----
            pT = small.tile([128, ES, COLS], bf16)
            id_ap = sub(id_v, 0, [128])
            for half in range(2):
                ps_t = pst.tile([128, 3, 128], bf16, tag="tr")
                for k in range(3):
                    s = half * 3 + k
                    nc.tensor.transpose(ps_t[:, k, :],
                                        pooled[:, s * 128:(s + 1) * 128], id_ap)
                nc.vector.tensor_copy(pT[:, half * 3:half * 3 + 3, :], ps_t)

            # ---- v-projection, directly in transposed layout ----
            oT_ps = pse.tile([128, 4, SLOTS], f32, tag="tp")
            for c4 in range(4):
                for hh in range(2):
                    h = 2 * c4 + hh
                    for s in range(ES):
                        nc.tensor.matmul(
                            oT_ps[64 * hh:64 * hh + 64, c4, :],
                            lhsT=sub(vT_v, s * D + h * DH, [DH]),
                            rhs=pT[:, s, h * SLOTS:(h + 1) * SLOTS],
                            start=(s == 0), stop=(s == ES - 1),
                            tile_position=(0, 64 * hh),
                        )

            # ---- out-projection with the bias folded in as a K=1 matmul;
            # oT copied in two halves on both engines so MMs start early ----
            oT = small.tile([128, 4, SLOTS], bf16)
            ps_f = ps8.tile([SLOTS, D], f32, tag="s8")
            nc.vector.tensor_copy(oT[:, 0:2, :], oT_ps[:, 0:2, :])
            nc.scalar.copy(oT[:, 2:4, :], oT_ps[:, 2:4, :])
            for s in range(4):
                nc.tensor.matmul(ps_f, lhsT=oT[:, s, :],
                                 rhs=sub(owT_v, s * D, [D]),
                                 start=(s == 0), stop=False)
            nc.tensor.matmul(ps_f, lhsT=sub(ones16, 0, [SLOTS], nparts=1),
                             rhs=sub(obr_v, 0, [D], nparts=1),
                             start=False, stop=True)

            res = small.tile([SLOTS, D], f32)
            nc.scalar.copy(res[:, 0:256], ps_f[:, 0:256])
            nc.vector.tensor_copy(res[:, 256:512], ps_f[:, 256:512])
            nc.sync.dma_start(out=out_d[:, :], in_=res)

    nc.finalize()
    return nc


def _host_prep(graph_emb, qry, q_w, k_w, v_w, in_b, out_w, out_b, ptr, batch):
    graph_emb = np.asarray(graph_emb, dtype=np.float32)
    qry = np.asarray(qry, dtype=np.float32)
    q_w = np.asarray(q_w, dtype=np.float32)
    k_w = np.asarray(k_w, dtype=np.float32)
    v_w = np.asarray(v_w, dtype=np.float32)
    in_b = np.asarray(in_b, dtype=np.float32)
    out_w = np.asarray(out_w, dtype=np.float32)
    out_b = np.asarray(out_b, dtype=np.float32)
    ptr = np.asarray(ptr).astype(np.int64)
    batch = np.asarray(batch).astype(np.int64)

    N = graph_emb.shape[0]
    B = len(ptr) - 1
    assert B <= NCORES * SLOTS, f"too many graphs: {B}"
    assert int(batch.max()) < B, "batch id out of ptr range"
    n_nodes = ptr[1:] - ptr[:-1]
    max_node = int(n_nodes.max()) + 1
    bs = int(batch.max()) + 1

    # --- mirror the reference's scatter semantics (jnp .at[] wraps negatives,
    # drops OOB, last write wins; valid mask is by slot index) ---
    pos = np.arange(N) - ptr[batch]
    m = np.where(pos < 0, pos + max_node, pos)
    part = (m >= 0) & (m < max_node) & (m < n_nodes[batch])
    idx = np.nonzero(part)[0]
    key = batch[idx] * max_node + m[idx]
    _, first_rev = np.unique(key[::-1], return_index=True)
    keep = idx[::-1][first_rev]
    keep.sort()
    kb = batch[keep]
    counts = np.bincount(kb, minlength=B)
    phantom = n_nodes.astype(np.float64) - counts  # valid-but-unfilled slots

    # --- q-side constant folding (qry is a model parameter) ---
    bq, bk, bv = in_b[:D], in_b[D:2 * D], in_b[2 * D:]
    scale = DH ** -0.5
    q = ((qry.reshape(-1)[-D:] @ q_w.T) + bq) * scale
    qh = q.reshape(H, DH)
    qk = np.stack([qh[h] @ k_w[h * DH:(h + 1) * DH, :] for h in range(H)])  # [8, E]
    ob_eff = out_b + out_w @ bv

    # --- balanced assignment: LPT greedy, 16 graphs per core ---
    order = np.argsort(-counts, kind="stable")
    slot_of = np.empty(B, dtype=np.int64)   # graph -> core*16+slot
    loads = np.zeros(NCORES, dtype=np.int64)
    nslots = np.zeros(NCORES, dtype=np.int64)
    for gi in order:
        cands = np.nonzero(nslots < SLOTS)[0]
        c = cands[np.argmin(loads[cands])]
        slot_of[gi] = c * SLOTS + nslots[c]
        nslots[c] += 1
        loads[c] += counts[gi]
    nc_pad = max(128, int(np.ceil(loads.max() / 128.0)) * 128)
    nch = nc_pad // 128

    nodes_of = [[] for _ in range(B)]
    for n in keep:
        nodes_of[batch[n]].append(n)

    # constants shared across cores
    hdr = np.zeros((128, 336), np.uint8)
    hdr[:, 0:48] = ((qk.T * QK_SCALE).astype(FP8)
                    .reshape(ES, 128, 8).transpose(1, 0, 2)
                    .reshape(128, 48).copy().view(np.uint8))
    hdr[:, 48:304] = np.eye(128, dtype=BF16).view(np.uint8)
    hdr[0, 304:336] = np.ones(SLOTS, BF16).view(np.uint8)
    c2 = np.zeros((128, 12292), np.uint8)
    c2[:, 0:6144] = np.ascontiguousarray(
        v_w.T.astype(BF16).reshape(ES, 128, D).transpose(1, 0, 2)
        .reshape(128, ES * D)).view(np.uint8)
    c2[:, 6144:10240] = np.ascontiguousarray(
        out_w.T.astype(BF16).reshape(4, 128, D).transpose(1, 0, 2)
        .reshape(128, 4 * D)).view(np.uint8)
    c2[0, 10244:10244 + 2 * D] = ob_eff.astype(BF16).view(np.uint8)

    in_maps = []
    for c in range(NCORES):
        rows = []
        ind16 = np.zeros((nc_pad, SLOTS), dtype=np.float32)
        ph_col = np.zeros((128, 1), dtype=np.float32)
        off = 0
        for s in range(SLOTS):
            gis = np.nonzero(slot_of == c * SLOTS + s)[0]
            if len(gis) == 0:
                continue
            gi = int(gis[0])
            ns = nodes_of[gi]
            rows.extend(ns)
            ind16[off:off + len(ns), s] = 1
            off += len(ns)
            for h in range(H):
                ph_col[h * SLOTS + s, 0] = phantom[gi]
        emb_c = np.zeros((nc_pad, E), dtype=np.float32)
        if rows:
            emb_c[:len(rows)] = graph_emb[np.asarray(rows)]

        # eb rows: per chunk [EW emb | 16 bf16 indT], partition p = n%128
        embw = np.zeros((nc_pad, EW), dtype=BF16)
        embw[:, 0:E] = emb_c.astype(BF16)
        embw[:, E] = 1.0
        ebrow = np.zeros((nc_pad, EBB), dtype=np.uint8)
        if EMB_FP8:
            ebrow[:, 0:EW] = embw.astype(np.float32).astype(FP8).view(np.uint8)
        else:
            ebrow[:, 0:EW * 2] = embw.view(np.uint8)
        ebrow[:, EMB_B:] = ind16.astype(BF16).view(np.uint8)
        ebc = ebrow.reshape(nch, 128, EBB)  # [ch, p, EBB]

        # embT fp8 [p, ch, s, n] for the score pass
        embT8 = (emb_c.astype(BF16).astype(np.float32).T.astype(FP8)
                 .reshape(ES, 128, nch, 128).transpose(1, 2, 0, 3))

        c2c = c2.copy()
        c2c[:, 10240:10244] = ph_col.astype(np.float32).view(np.uint8)
        imap = {"c2": c2c}
        cc = 0
        for g, J in enumerate(_jlist(nch)):
            h = 336 if g == 0 else 0
            blk = np.empty((128, h + J * (ETB + EBB)), np.uint8)
            if g == 0:
                blk[:, 0:336] = hdr
            blk[:, h:h + J * ETB] = np.ascontiguousarray(
                embT8[:, cc:cc + J].transpose(0, 2, 1, 3)  # [p, s, j, n]
            ).reshape(128, J * ETB).view(np.uint8)
            blk[:, h + J * ETB:] = np.ascontiguousarray(
                ebc[cc:cc + J].transpose(1, 0, 2)).reshape(128, J * EBB)
            imap[f"g{g}"] = blk
            cc += J
        in_maps.append(imap)

    meta = {
        "bs": bs,
        "slot_of": slot_of,
        "n_nodes": n_nodes,
        "nc_pad": nch,
    }
    return in_maps, meta


def _assemble(results, meta):
    bs = meta["bs"]
    slot_of = meta["slot_of"]
    n_nodes = meta["n_nodes"]
    out = np.empty((bs, D), dtype=np.float32)
    for b in range(bs):
        sl = int(slot_of[b])
        out[b] = results[sl // SLOTS]["out"][sl % SLOTS]
        if n_nodes[b] <= 0:
            out[b] = np.nan
    return out


def kernel(graph_emb, qry, q_w, k_w, v_w, in_b, out_w, out_b, ptr, batch):
    from concourse.bass_utils import run_bass_kernel_spmd

    in_maps, meta = _host_prep(graph_emb, qry, q_w, k_w, v_w, in_b, out_w,
                               out_b, ptr, batch)
    nch = meta["nc_pad"]
    if nch not in _prog_cache:
        _prog_cache[nch] = _build_program(nch)
    nc = _prog_cache[nch]
    res = run_bass_kernel_spmd(nc, in_maps, list(range(NCORES)))
    return _assemble(res.results, meta)


# revision 34
# speedup vs baseline: 1.1676x; 1.1676x over previous
"""AttentionPooling (ragged segment attention) on 8 Trainium2 NeuronCores.

Full inputs in, full output out. Strategy (data-parallel over graphs):
  - 128 graphs are LPT-balanced 16-per-core across 8 cores; each core gets
    its graphs' node embeddings (zero-padded to a multiple of 128 rows).
  - The single shared query is a model parameter, so the q-side is constant-
    folded on the host:  qk8[e,h] = sum_d q_scaled[h,d]*k_w[h*64+d,e].
  - Nodes stream through in groups of up to 4 x 128-node chunks (small
    groups at both ends for pipeline fill/drain), one contiguous DMA per
    group carrying both layouts: embT fp8 [e,n] for scores, emb fp8 [n,e]
    (+ ones column for the colsum, + per-node slot indicator) for pooling.
  - On device (per core), per group (cols c = h*16 + s):
      s8T[h, n]  = sum_e qk8[e,h] * embT[e,n]      (PE fp8, qk stationary)
      e8T[h, n]  = exp(s8T / QK_SCALE)             (ACT)
      e8[n, h]   = PE-transpose(e8T)               (per chunk [8,128]->[128,8])
      em[n, c]   = e8[n, h] * indT[n, s]           (DVE broadcast-mul)
      pooled[c,0:776] += em^T @ [emb | 1 | 0pad]   (PE accum; col 768=colsum)
    Softmax exp(qb) factors cancel between numerator and denominator; the
    phantom-slot correction is a host-computed count added to the colsum.
  - Tail: normalize by colsum (ACT+DVE halves), PE-transpose pooled,
    block-diag v-proj (2-head col-tiling), out-proj with the bias folded in
    as a K=1 matmul.  Host gathers the 8x[16,512] results into [bs, 512].
"""

import numpy as np
import ml_dtypes

BF16 = ml_dtypes.bfloat16
FP8 = ml_dtypes.float8_e3m4
QK_SCALE = 128.0
E = 768
EW = 776            # emb row + ones col + pad  (EW-768-1 zeros)
D = 512
H = 8
DH = 64
NCORES = 8
SLOTS = 16          # graphs per core
COLS = 128          # H * SLOTS
ES = E // 128       # 6 E-slices of 128
EMB_FP8 = True                # pool rhs in fp8 e3m4 (halves the emb stream)
ETB = 6 * 128                 # embT bytes per chunk per partition
EMB_B = EW if EMB_FP8 else EW * 2     # emb bytes per chunk per partition
EBB = EMB_B + SLOTS * 2       # emb+ind bytes per chunk per partition

_prog_cache = {}


def _jlist(nch):
    """Chunks per group: small groups at both ends for fast pipeline
    fill (first scores start after a 98KB DMA, not 393KB) and drain."""
    if nch <= 7:
        js = []
        r = nch
        while r > 0:
            j = min(2, r)
            js.append(j)
            r -= j
        return js
    js = [1, 1, 2]
    rem = nch - 7
    js += [4] * (rem // 4)
    if rem % 4:
        js.append(rem % 4)
    js += [2, 1]
    return js


def _build_program(nch):
    import concourse.bacc as bacc
    import concourse.tile as tile
    import concourse.mybir as mybir
    from concourse.bass import AP

    f32 = mybir.dt.float32
    bf16 = mybir.dt.bfloat16
    f8 = mybir.dt.float8e3
    u8 = mybir.dt.uint8
    AF = mybir.ActivationFunctionType

    J_of = _jlist(nch)
    ngrp = len(J_of)
    ch0 = np.concatenate([[0], np.cumsum(J_of)]).astype(int)  # first chunk of g

    nc = bacc.Bacc(None, target_bir_lowering=False)

    # One contiguous DRAM param per group (sequential HBM reads):
    #   [embT fp8 [s(6), j(J), n(128)] @ p=e%128 | emb fp8 [j, EW] +
    #    indT bf16 [j, 16] @ p=n%128]
    # Group 0 carries a 336B header: qk8 fp8 48B | ident bf16 256B |
    # ones16 bf16 32B -- one DMA covers all launch-critical bytes.
    HDR = 336
    g_d = [nc.declare_dram_parameter(
        f"g{g}", [128, (HDR if g == 0 else 0) + J_of[g] * (ETB + EBB)],
        u8, isOutput=False) for g in range(ngrp)]
    # consts2: vT bf16 [6,512]=6144B | owT bf16 [4,512]=4096B | ph f32 4B |
    #          ob row f32 2048B (partition 0 only)
    c2_d = nc.declare_dram_parameter("c2", [128, 6144 + 4096 + 4 + 2048], u8,
                                     isOutput=False)
    out_d = nc.declare_dram_parameter("out", [SLOTS, D], f32, isOutput=True)

    def bview(tile_, byte_off, dt, shape, nparts=128):
        """AP viewing bytes [byte_off:] of a uint8 tile as dtype with the
        given free-dim shape (row-major, contiguous)."""
        esz = mybir.dt.size(dt)
        assert byte_off % esz == 0
        base = tile_[:, :].bitcast(dt)
        strides = []
        acc = 1
        for s in reversed(shape):
            strides.append(acc)
            acc *= s
        strides = strides[::-1]
        newap = [[base.ap[0][0], nparts]] + [[st, sz]
                                             for st, sz in zip(strides, shape)]
        return AP(base.tensor, base.offset + byte_off // esz, newap)

    def sub(ap, elem_off, shape, nparts=None):
        """Sub-AP at elem_off (in ap dtype elements) with contiguous shape."""
        strides = []
        acc = 1
        for s in reversed(shape):
            strides.append(acc)
            acc *= s
        strides = strides[::-1]
        p = [ap.ap[0][0], nparts if nparts is not None else ap.ap[0][1]]
        newap = [p] + [[st, sz] for st, sz in zip(strides, shape)]
        return AP(ap.tensor, ap.offset + elem_off, newap)

    with tile.TileContext(nc) as tc:
        with (
            tc.tile_pool(name="const", bufs=1) as const,
            tc.tile_pool(name="gb_p", bufs=6) as gb_p,
            tc.tile_pool(name="e8_p", bufs=2) as e8_p,
            tc.tile_pool(name="em_p", bufs=3) as em_p,
            tc.tile_pool(name="small", bufs=1) as small,
            tc.tile_pool(name="ps8", bufs=2, space="PSUM") as ps8,
            tc.tile_pool(name="pse", bufs=2, space="PSUM") as pse,
            tc.tile_pool(name="pacc", bufs=1, space="PSUM") as pacc,
            tc.tile_pool(name="pst", bufs=2, space="PSUM") as pst,
        ):
            # ---- group 0 (with header) goes through the const pool ----
            g0_sb = const.tile([128, HDR + J_of[0] * (ETB + EBB)], u8)
            nc.sync.dma_start(out=g0_sb, in_=g_d[0][:, :])
            qk_v = bview(g0_sb, 0, f8, [ES * 8])
            id_v = bview(g0_sb, 48, bf16, [128])
            id8 = sub(id_v, 0, [8], nparts=8)
            ones16 = bview(g0_sb, 304, bf16, [SLOTS], nparts=1)

            # persistent accumulator: pooled [COLS, 776] f32 (col 768 = colsum)
            ps_pool = pacc.tile([COLS, EW], f32)

            gbs = {0: (g0_sb, HDR)}

            def load_g(g):
                J = J_of[g]
                t = gb_p.tile([128, 4 * (ETB + EBB)], u8, tag="gb")
                nc.sync.dma_start(out=t[:, 0:J * (ETB + EBB)], in_=g_d[g][:, :])
                gbs[g] = (t, 0)

            def emit_scores(g):
                """s8T[h, J*128] = sum_s qk[:,s,:].T @ et[:, s, :]; then exp."""
                J = J_of[g]
                gt, gb0 = gbs[g]
                et = bview(gt, gb0, f8, [ES, J * 128])
                ps_s = ps8.tile([8, 512], f32, tag="s8")
                for s in range(ES):
                    nc.tensor.matmul(ps_s[:, 0:J * 128],
                                     lhsT=sub(qk_v, s * 8, [8]),
                                     rhs=sub(et, s * J * 128, [J * 128]),
                                     start=(s == 0), stop=(s == ES - 1))
                e8T = e8_p.tile([8, 512], bf16, tag="e8")
                nc.scalar.activation(out=e8T[:, 0:J * 128], in_=ps_s[:, 0:J * 128],
                                     func=AF.Exp, scale=1.0 / QK_SCALE)
                return e8T

            def emit_em(g, e8T):
                """e8 = transpose(e8T) per chunk; em[n,(h,s)] = e8*indT."""
                J = J_of[g]
                e8_ps = pse.tile([128, 4, 8], bf16, tag="tp")
                for j in range(J):
                    nc.tensor.transpose(e8_ps[:, j, :],
                                        e8T[:, j * 128:(j + 1) * 128], id8)
                em = em_p.tile([128, 4, H, SLOTS], bf16, tag="em")
                gt, gb0 = gbs[g]
                ind = bview(gt, gb0 + J * ETB + EMB_B, bf16, [J, EBB // 2])
                a = e8_ps[:, 0:J, :]
                bc_e8 = AP(a.tensor, a.offset,
                           [list(a.ap[0]), list(a.ap[1]), list(a.ap[2]),
                            [0, SLOTS]])
                bc_ind = AP(ind.tensor, ind.offset,
                            [list(ind.ap[0]), list(ind.ap[1]), [0, H],
                             [1, SLOTS]])
                nc.vector.tensor_mul(em[:, 0:J, :, :], bc_e8, bc_ind)
                return em

            def emit_pool(g, em):
                J = J_of[g]
                edt = f8 if EMB_FP8 else bf16
                esz = 1 if EMB_FP8 else 2
                gt, gb0 = gbs.pop(g)
                assert (gb0 + J * ETB) % esz == 0
                emb = bview(gt, gb0 + J * ETB, edt, [J, EBB // esz])
                for j in range(J):
                    ch = ch0[g] + j
                    st = (ch == 0)
                    sp = (ch == nch - 1)
                    lhsT = em[:, j, :, :]
                    nc.tensor.matmul(ps_pool[:, 512:EW], lhsT=lhsT,
                                     rhs=sub(emb, j * (EBB // esz) + 512,
                                             [EW - 512]),
                                     start=st, stop=sp)
                    nc.tensor.matmul(ps_pool[:, 0:512], lhsT=lhsT,
                                     rhs=sub(emb, j * (EBB // esz), [512]),
                                     start=st, stop=sp)

            # ---- software pipeline ----
            # stages per g: scores(g) | transpose+em(g-1) | pool(g-2)
            for g in range(1, min(3, ngrp)):
                load_g(g)
            c2_sb = const.tile([128, 12292], u8)
            c2_done = False
            e8s, ems = {}, {}
            for g in range(ngrp):
                e8s[g] = emit_scores(g)
                if g >= 1:
                    ems[g - 1] = emit_em(g - 1, e8s.pop(g - 1))
                if g + 3 < ngrp:
                    load_g(g + 3)
                elif not c2_done:
                    nc.sync.dma_start(out=c2_sb, in_=c2_d[:, :])
                    c2_done = True
                if g >= 2:
                    emit_pool(g - 2, ems.pop(g - 2))
            ems[ngrp - 1] = emit_em(ngrp - 1, e8s.pop(ngrp - 1))
            for g in sorted(ems):
                emit_pool(g, ems.pop(g))

            vT_v = bview(c2_sb, 0, bf16, [ES * D])
            owT_v = bview(c2_sb, 6144, bf16, [4 * D])
            ph_v = bview(c2_sb, 10240, f32, [1])
            obr_v = bview(c2_sb, 10244, bf16, [D], nparts=1)

            # ---- normalize: pooled[c, e] / (colsum[c] + phantom[c]);
            # per-slice on alternating engines, PE transpose right behind ----
            cs_sb = small.tile([COLS, 1], f32)
            nc.vector.tensor_add(cs_sb, ps_pool[:, 768:769], ph_v)
            rec_sb = small.tile([COLS, 1], f32)
            nc.vector.reciprocal(rec_sb, cs_sb)
            pooled = small.tile([COLS, E], bf16)
            nc.scalar.activation(out=pooled[:, 0:384], in_=ps_pool[:, 0:384],
                                 func=AF.Copy, scale=rec_sb[:, :])
            nc.vector.tensor_scalar_mul(pooled[:, 384:768],
                                        in0=ps_pool[:, 384:768],
                                        scalar1=rec_sb)
            # ---- pooledT via PE transposes, 2 PSUM tiles, 2 copies ----
            pT = small.tile([128, ES, COLS], bf16)
            id_ap = sub(id_v, 0, [128])
            for half in range(2):
                ps_t = pst.tile([128, 3, 128], bf16, tag="tr")
                for k in range(3):
                    s = half * 3 + k
                    nc.tensor.transpose(ps_t[:, k, :],
                                        pooled[:, s * 128:(s + 1) * 128], id_ap)
                nc.vector.tensor_copy(pT[:, half * 3:half * 3 + 3, :], ps_t)

            # ---- v-projection, directly in transposed layout ----
            oT_ps = pse.tile([128, 4, SLOTS], f32, tag="tp")
            for c4 in range(4):
                for hh in range(2):
                    h = 2 * c4 + hh
                    for s in range(ES):
                        nc.tensor.matmul(
                            oT_ps[64 * hh:64 * hh + 64, c4, :],
                            lhsT=sub(vT_v, s * D + h * DH, [DH]),
                            rhs=pT[:, s, h * SLOTS:(h + 1) * SLOTS],
                            start=(s == 0), stop=(s == ES - 1),
                            tile_position=(0, 64 * hh),
                        )

            # ---- out-projection with the bias folded in as a K=1 matmul;
            # oT copied in two halves on both engines so MMs start early ----
            oT = small.tile([128, 4, SLOTS], bf16)
            ps_f = ps8.tile([SLOTS, D], f32, tag="s8")
            nc.vector.tensor_copy(oT[:, 0:2, :], oT_ps[:, 0:2, :])
            nc.scalar.copy(oT[:, 2:4, :], oT_ps[:, 2:4, :])
            for s in range(4):
                nc.tensor.matmul(ps_f, lhsT=oT[:, s, :],
                                 rhs=sub(owT_v, s * D, [D]),
                                 start=(s == 0), stop=False)
            nc.tensor.matmul(ps_f, lhsT=sub(ones16, 0, [SLOTS], nparts=1),
                             rhs=sub(obr_v, 0, [D], nparts=1),
                             start=False, stop=True)

            res = small.tile([SLOTS, D], f32)
            nc.scalar.copy(res[:, 0:256], ps_f[:, 0:256])
            nc.vector.tensor_copy(res[:, 256:512], ps_f[:, 256:512])
            nc.sync.dma_start(out=out_d[:, :], in_=res)

    nc.finalize()
    return nc


def _host_prep(graph_emb, qry, q_w, k_w, v_w, in_b, out_w, out_b, ptr, batch):
    graph_emb = np.asarray(graph_emb, dtype=np.float32)
    qry = np.asarray(qry, dtype=np.float32)
    q_w = np.asarray(q_w, dtype=np.float32)
    k_w = np.asarray(k_w, dtype=np.float32)
    v_w = np.asarray(v_w, dtype=np.float32)
    in_b = np.asarray(in_b, dtype=np.float32)
    out_w = np.asarray(out_w, dtype=np.float32)
    out_b = np.asarray(out_b, dtype=np.float32)
    ptr = np.asarray(ptr).astype(np.int64)
    batch = np.asarray(batch).astype(np.int64)

    N = graph_emb.shape[0]
    B = len(ptr) - 1
    assert B <= NCORES * SLOTS, f"too many graphs: {B}"
    assert int(batch.max()) < B, "batch id out of ptr range"
    n_nodes = ptr[1:] - ptr[:-1]
    max_node = int(n_nodes.max()) + 1
    bs = int(batch.max()) + 1

    # --- mirror the reference's scatter semantics (jnp .at[] wraps negatives,
    # drops OOB, last write wins; valid mask is by slot index) ---
    pos = np.arange(N) - ptr[batch]
    m = np.where(pos < 0, pos + max_node, pos)
    part = (m >= 0) & (m < max_node) & (m < n_nodes[batch])
    idx = np.nonzero(part)[0]
    key = batch[idx] * max_node + m[idx]
    _, first_rev = np.unique(key[::-1], return_index=True)
    keep = idx[::-1][first_rev]
    keep.sort()
    kb = batch[keep]
    counts = np.bincount(kb, minlength=B)
    phantom = n_nodes.astype(np.float64) - counts  # valid-but-unfilled slots

    # --- q-side constant folding (qry is a model parameter) ---
    bq, bk, bv = in_b[:D], in_b[D:2 * D], in_b[2 * D:]
    scale = DH ** -0.5
    q = ((qry.reshape(-1)[-D:] @ q_w.T) + bq) * scale
    qh = q.reshape(H, DH)
    qk = np.stack([qh[h] @ k_w[h * DH:(h + 1) * DH, :] for h in range(H)])  # [8, E]
    ob_eff = out_b + out_w @ bv

    # --- balanced assignment: LPT greedy, 16 graphs per core ---
    order = np.argsort(-counts, kind="stable")
    slot_of = np.empty(B, dtype=np.int64)   # graph -> core*16+slot
    loads = np.zeros(NCORES, dtype=np.int64)
    nslots = np.zeros(NCORES, dtype=np.int64)
    for gi in order:
        cands = np.nonzero(nslots < SLOTS)[0]
        c = cands[np.argmin(loads[cands])]
        slot_of[gi] = c * SLOTS + nslots[c]
        nslots[c] += 1
        loads[c] += counts[gi]
    nc_pad = max(128, int(np.ceil(loads.max() / 128.0)) * 128)
    nch = nc_pad // 128

    nodes_of = [[] for _ in range(B)]
    for n in keep:
        nodes_of[batch[n]].append(n)

    # constants shared across cores
    hdr = np.zeros((128, 336), np.uint8)
    hdr[:, 0:48] = ((qk.T * QK_SCALE).astype(FP8)
                    .reshape(ES, 128, 8).transpose(1, 0, 2)
                    .reshape(128, 48).copy().view(np.uint8))
    hdr[:, 48:304] = np.eye(128, dtype=BF16).view(np.uint8)
    hdr[0, 304:336] = np.ones(SLOTS, BF16).view(np.uint8)
    c2 = np.zeros((128, 12292), np.uint8)
    c2[:, 0:6144] = np.ascontiguousarray(
        v_w.T.astype(BF16).reshape(ES, 128, D).transpose(1, 0, 2)
        .reshape(128, ES * D)).view(np.uint8)
    c2[:, 6144:10240] = np.ascontiguousarray(
        out_w.T.astype(BF16).reshape(4, 128, D).transpose(1, 0, 2)
        .reshape(128, 4 * D)).view(np.uint8)
    c2[0, 10244:10244 + 2 * D] = ob_eff.astype(BF16).view(np.uint8)

    in_maps = []
    for c in range(NCORES):
        rows = []
        ind16 = np.zeros((nc_pad, SLOTS), dtype=np.float32)
        ph_col = np.zeros((128, 1), dtype=np.float32)
        off = 0
        for s in range(SLOTS):
            gis = np.nonzero(slot_of == c * SLOTS + s)[0]
            if len(gis) == 0:
                continue
            gi = int(gis[0])
            ns = nodes_of[gi]
            rows.extend(ns)
            ind16[off:off + len(ns), s] = 1
            off += len(ns)
            for h in range(H):
                ph_col[h * SLOTS + s, 0] = phantom[gi]
        emb_c = np.zeros((nc_pad, E), dtype=np.float32)
        if rows:
            emb_c[:len(rows)] = graph_emb[np.asarray(rows)]

        # eb rows: per chunk [EW emb | 16 bf16 indT], partition p = n%128
        embw = np.zeros((nc_pad, EW), dtype=BF16)
        embw[:, 0:E] = emb_c.astype(BF16)
        embw[:, E] = 1.0
        ebrow = np.zeros((nc_pad, EBB), dtype=np.uint8)
        if EMB_FP8:
            ebrow[:, 0:EW] = embw.astype(np.float32).astype(FP8).view(np.uint8)
        else:
            ebrow[:, 0:EW * 2] = embw.view(np.uint8)
        ebrow[:, EMB_B:] = ind16.astype(BF16).view(np.uint8)
        ebc = ebrow.reshape(nch, 128, EBB)  # [ch, p, EBB]

        # embT fp8 [p, ch, s, n] for the score pass
        embT8 = (emb_c.astype(BF16).astype(np.float32).T.astype(FP8)
                 .reshape(ES, 128, nch, 128).transpose(1, 2, 0, 3))

        c2c = c2.copy()
        c2c[:, 10240:10244] = ph_col.astype(np.float32).view(np.uint8)
        imap = {"c2": c2c}
        cc = 0
        for g, J in enumerate(_jlist(nch)):
            h = 336 if g == 0 else 0
            blk = np.empty((128, h + J * (ETB + EBB)), np.uint8)
            if g == 0:
                blk[:, 0:336] = hdr
            blk[:, h:h + J * ETB] = np.ascontiguousarray(
                embT8[:, cc:cc + J].transpose(0, 2, 1, 3)  # [p, s, j, n]
            ).reshape(128, J * ETB).view(np.uint8)
            blk[:, h + J * ETB:] = np.ascontiguousarray(
                ebc[cc:cc + J].transpose(1, 0, 2)).reshape(128, J * EBB)
            imap[f"g{g}"] = blk
            cc += J
        in_maps.append(imap)

    meta = {
        "bs": bs,
        "slot_of": slot_of,
        "n_nodes": n_nodes,
        "nc_pad": nch,
    }
    return in_maps, meta


def _assemble(results, meta):
    bs = meta["bs"]
    slot_of = meta["slot_of"]
    n_nodes = meta["n_nodes"]
    out = np.empty((bs, D), dtype=np.float32)
    for b in range(bs):
        sl = int(slot_of[b])
        out[b] = results[sl // SLOTS]["out"][sl % SLOTS]
        if n_nodes[b] <= 0:
            out[b] = np.nan
    return out


def kernel(graph_emb, qry, q_w, k_w, v_w, in_b, out_w, out_b, ptr, batch):
    from concourse.bass_utils import run_bass_kernel_spmd

    in_maps, meta = _host_prep(graph_emb, qry, q_w, k_w, v_w, in_b, out_w,
                               out_b, ptr, batch)
    nch = meta["nc_pad"]
    if nch not in _prog_cache:
        _prog_cache[nch] = _build_program(nch)
    nc = _prog_cache[nch]
    res = run_bass_kernel_spmd(nc, in_maps, list(range(NCORES)))
    return _assemble(res.results, meta)
